# revision 69
# baseline (speedup 1.0000x reference)
"""GPT-2 style causal attention block (B=4, S=2048, E=1024, H=16, D=64) on
8 TRN2 NeuronCores.

Sharding: batch(4) x head-half(2) -> 8 cores, zero on-device communication.
Core c handles batch b=c//2 and heads h0=(c%2)*8 .. h0+7. Each core computes
its qkv column block, attention for its 8 heads, and a partial c_proj
(its 512 rows of w_proj); core pairs are summed on the host.

Key structure (per core):
  X^T [E, S] host-pre-transposed bf16.
  Q,K are produced in fp8(e4m3) with a paired-slab layout [32, 2, S] per
  4-head group (w_attn columns reordered on the host so each psum group is
  one [128, 512] tile): scores run as fp8 DoubleRow matmuls at 0.5
  cycles/column, contraction d=64 as 2 slabs of 32.
  exp on ACT (scale 1/8 folded) into a rotating pt store [kt, q]; causal
  diagonal via gpsimd affine_select.
  attn@V is *flipped*: per (head, q-block) bursts with the pt block as the
  (free) stationary and [V_h | ones] as the 65-wide moving operand ->
  65 cycles per 128x128 block, denominator in column 64.
  Normalize on DVE (recip + per-partition scalar mul) into per-pair [128q,
  128hd] staging tiles, then DMA-transpose (xbar) into A^T [hd, q] - no PE
  or DVE cycles spent on the transpose.
  c_proj tiles 0-7 after the c0h7 pass; tiles 8-15 split ct0-2 (after
  c1h5) + ct3 per q-block inside the last pass, partials staged in bf16;
  the final adds alternate DVE and ACT-copy+Pool so no engine serializes
  the drain.

Scheduling: head passes are interleaved across the two q-chunks
(c0h0-3, c1h0, c1h1, c0h4, ...) so the ACT engine - the exp chain is its
critical path - stays fed from ~10us to the end instead of a PE-bound
first half and ACT-bound second half. All qkv/V groups are emitted up
front in ACT-unlock order (qkA01, qkA23, V-quadA, qkB01, V-quadB, qkB23);
attention bodies are priority-boosted over them. pt slots rotate by 17
(mod 24) per pass so a pass's exps only collide with bursts >=1 pass
back. PSUM: scores 2x2 banks, AV bursts 2x1, fillers 2x1.
"""

import re

import ml_dtypes
import numpy as np

import concourse.mybir as mybir
import concourse.tile as tile
from concourse import bacc
from concourse.bass_utils import run_bass_kernel_spmd
from concourse.vector_clock import ScopedClock

F32 = mybir.dt.float32
BF16 = mybir.dt.bfloat16
FP8 = mybir.dt.float8e4
BF16_NP = ml_dtypes.bfloat16
FP8_NP = ml_dtypes.float8_e4m3fn
AF = mybir.ActivationFunctionType
DR = mybir.MatmulPerfMode.DoubleRow

S = 2048          # sequence length (per batch)
E = 1024          # embedding dim
HL = 8            # heads per core
D = 64            # head dim
TT = S // 128     # 16 token tiles
ET = E // 128     # 8 embedding tiles
NCH = S // 1024   # 2 q-chunks of 1024
PTS = 24          # pt slot count; advance 17 per pass (coprime) so adjacent
                  # passes never collide and older overlaps spread thin
PRIO_OFFSET = 800  # attention body scheduled ahead of filler work

# wqk block order (host column layout): QA0 QA1 KA0 KA1 QB0 QB1 KB0 KB1
# grp: 0=QA 1=KA 2=QB 3=KB ; slab: d 0:32 / 32:64 per head
BLK_GRP = [0, 0, 1, 1, 2, 2, 3, 3]
BLK_SLAB = [0, 1, 0, 1, 0, 1, 0, 1]


def _install_drain_fix():
    """walrus in this container rejects the Tile kernel-tail Drain when it
    carries all semaphore waits on one instruction ("Too many sync wait
    commands"). Emit one wait_ge per semaphore, then a bare drain."""
    if getattr(tile.TileContext, "_drain_fix_installed", False):
        return

    def _split_drain_and_barrier(self, tick_clock, wait_clock):
        nc = self.nc
        probe = mybir.InstDrain(
            name="probe-drain", engine=mybir.EngineType.SP, ins=[], outs=[]
        )
        wait_clock.add_sem_waits(probe, ScopedClock({None: tick_clock.global_clock}))
        waits = re.findall(r"wait:S\[([A-Za-z0-9_]+)\]>=(\d+)", probe.concise())
        handles = {h.name: h for h in self.sems.allocated().values()}
        for name, val in waits:
            nc.sync.wait_ge(handles[name], int(val))
        nc.sync.drain()
        nc.all_engine_barrier()
        popped = nc._tile_sem_poison_stack.pop()
        assert popped is self._sem_poison
        nc.clear_and_free_semaphores(list(self.sems.allocated().values()))
        nc.all_engine_barrier()

    tile.TileContext._drain_and_barrier = _split_drain_and_barrier
    tile.TileContext._drain_fix_installed = True


def _emit(nc, tc, ctx):
    xt_d = nc.declare_dram_parameter("xt", [E, S], BF16, isOutput=False)
    # wqk host-side block-major: [block, partition, et, col] so one block =
    # one contiguous 256KB DMA with 2KB descriptor lines
    wqk_d = nc.declare_dram_parameter("wqk", [8, 128, ET, 128], BF16, isOutput=False)
    wva_d = nc.declare_dram_parameter("wva", [E, 512], BF16, isOutput=False)
    wp_d = nc.declare_dram_parameter("wp", [512, E], BF16, isOutput=False)
    bqk_d = nc.declare_dram_parameter("bqk", [128, 8], F32, isOutput=False)
    bva_d = nc.declare_dram_parameter("bva", [1, 512], BF16, isOutput=False)
    bp_d = nc.declare_dram_parameter("bp", [1, E], BF16, isOutput=False)
    out_d = nc.declare_dram_parameter("out", [S, E], BF16, isOutput=True)

    consts = ctx.enter_context(tc.tile_pool(name="consts", bufs=1))
    statics = ctx.enter_context(tc.tile_pool(name="statics", bufs=1))
    anp = ctx.enter_context(tc.tile_pool(name="anp", bufs=20))
    rp = ctx.enter_context(tc.tile_pool(name="rp", bufs=4))
    yp = ctx.enter_context(tc.tile_pool(name="yp", bufs=3))
    stg = ctx.enter_context(tc.tile_pool(name="stg", bufs=2))
    # PSUM budget (8 banks): scores 2x[128,1024]=4, AV 2x[128,512(:65)]=2,
    # fillers 2x[128,512]=2
    psS = ctx.enter_context(tc.tile_pool(name="psS", bufs=2, space="PSUM"))
    psA = ctx.enter_context(tc.tile_pool(name="psA", bufs=2, space="PSUM"))
    psF = ctx.enter_context(tc.tile_pool(name="psF", bufs=2, space="PSUM"))

    # ---- static SBUF tiles ----
    xt_sb = statics.tile([128, ET, S], BF16)
    wqk_sb = statics.tile([128, 8, ET, 128], BF16)
    wva_sb = statics.tile([128, ET, 512], BF16)
    wp_sb = statics.tile([128, 4, E], BF16)
    # Q,K fp8 paired-slab store: [part 32m, grp, slab, col]
    qk8_sb = statics.tile([128, 4, 2, S], FP8)
    va_sb = statics.tile([128, TT, HL * 65], BF16)
    pt_sb = statics.tile([128, PTS, 1024], BF16)
    at_sb = statics.tile([128, 4, S], BF16)  # A^T: rows c=h*64+d, cols t
    yh_sb = statics.tile([128, 8, E], BF16)  # tail (tiles 8-15) ct0-2 partial

    # ---- front DMA: the DMA engines are a single serialized ~360GB/s
    # resource and issue overhead is per-queue, so spread across three
    # queues: SP carries xt (tch-major pieces so chunk-0 unlocks first),
    # ACT (idle during the ramp) carries the wqk blocks, Pool carries
    # wva/wp. Biases first (tiny, gate the qk8 copies). ----
    bqk_sb = consts.tile([128, 8], F32)
    nc.sync.dma_start(out=bqk_sb, in_=bqk_d[:])
    for b in range(4):
        nc.scalar.dma_start(out=wqk_sb[:, b], in_=wqk_d[b])

    for et in range(ET):
        nc.sync.dma_start(
            out=xt_sb[:, et, 0:512], in_=xt_d[et * 128 : (et + 1) * 128, 0:512]
        )
    for et in range(ET):
        nc.sync.dma_start(
            out=xt_sb[:, et, 512:1024],
            in_=xt_d[et * 128 : (et + 1) * 128, 512:1024],
        )
    for et in range(ET):
        nc.sync.dma_start(
            out=xt_sb[:, et, 1024:1536],
            in_=xt_d[et * 128 : (et + 1) * 128, 1024:1536],
        )
    for et in range(ET):
        nc.gpsimd.dma_start(
            out=wva_sb[:, et, :], in_=wva_d[et * 128 : (et + 1) * 128, :]
        )
    for b in range(4, 8):
        nc.gpsimd.dma_start(out=wqk_sb[:, b], in_=wqk_d[b])
    bva_st = consts.tile([1, 512], BF16)
    nc.gpsimd.dma_start(out=bva_st, in_=bva_d[:])
    bva_bc = consts.tile([128, 512], BF16)
    nc.gpsimd.partition_broadcast(out_ap=bva_bc[:], in_ap=bva_st[:])
    bp_st = consts.tile([1, E], BF16)
    nc.gpsimd.dma_start(out=bp_st, in_=bp_d[:])
    bp_bc = consts.tile([128, E], BF16)
    nc.gpsimd.partition_broadcast(out_ap=bp_bc[:], in_ap=bp_st[:])
    for et in range(ET):
        nc.sync.dma_start(
            out=xt_sb[:, et, 1536:2048],
            in_=xt_d[et * 128 : (et + 1) * 128, 1536:2048],
        )
    for ct in range(4):
        nc.gpsimd.dma_start(out=wp_sb[:, ct, :], in_=wp_d[ct * 128 : (ct + 1) * 128, :])

    # per-head ones columns (position 64 of each 65-wide head block)
    nc.gpsimd.memset(
        va_sb[:, :, :].rearrange("p i (h c) -> p i h c", c=65)[:, :, :, D : D + 1],
        1.0,
    )

    # ---- qkv emitters ----
    def emit_qk(b, tch):
        g, sl = BLK_GRP[b], BLK_SLAB[b]
        p = psF.tile([128, 512], F32, tag="f")
        for et in range(ET):
            nc.tensor.matmul(
                p,
                wqk_sb[:, b, et, :],
                xt_sb[:, et, tch * 512 : (tch + 1) * 512],
                start=(et == 0),
                stop=(et == ET - 1),
            )
        nc.vector.tensor_scalar_add(
            qk8_sb[:, g, sl, tch * 512 : (tch + 1) * 512], p, bqk_sb[:, b : b + 1]
        )

    def emit_v(i, q):  # token tile i, head-quad q (heads 4q..4q+3)
        p = psF.tile([128, 512], F32, tag="f")
        for et in range(ET):
            nc.tensor.matmul(
                p[:, 0:256],
                xt_sb[:, et, i * 128 : (i + 1) * 128],
                wva_sb[:, et, q * 256 : (q + 1) * 256],
                start=(et == 0),
                stop=(et == ET - 1),
            )
        nc.vector.tensor_add(
            va_sb[:, i, 260 * q : 260 * (q + 1)].rearrange(
                "p (h c) -> p h c", c=65
            )[:, :, 0:D],
            p[:, 0:256].rearrange("p (h c) -> p h c", c=D),
            bva_bc[:, q * 256 : (q + 1) * 256].rearrange("p (h c) -> p h c", c=D),
        )

    def emit_cproj(i):  # tiles 0-7: full ct accumulation
        ysb = yp.tile([128, E], BF16, tag="y")
        for ech in range(2):
            p = psF.tile([128, 512], F32, tag="f")
            for ct in range(4):
                nc.tensor.matmul(
                    p,
                    at_sb[:, ct, i * 128 : (i + 1) * 128],
                    wp_sb[:, ct, ech * 512 : (ech + 1) * 512],
                    start=(ct == 0),
                    stop=(ct == 3),
                )
            nc.vector.tensor_add(
                ysb[:, ech * 512 : (ech + 1) * 512],
                p,
                bp_bc[:, ech * 512 : (ech + 1) * 512],
            )
        nc.sync.dma_start(out=out_d[i * 128 : (i + 1) * 128, :], in_=ysb)

    def emit_tail_half(i):  # tiles 8-15, ct 0..2 (+bias) into yh_sb
        for ech in range(2):
            p = psF.tile([128, 512], F32, tag="f")
            for ct in range(3):
                nc.tensor.matmul(
                    p,
                    at_sb[:, ct, i * 128 : (i + 1) * 128],
                    wp_sb[:, ct, ech * 512 : (ech + 1) * 512],
                    start=(ct == 0),
                    stop=(ct == 2),
                )
            nc.vector.tensor_add(
                yh_sb[:, i - 8, ech * 512 : (ech + 1) * 512],
                p,
                bp_bc[:, ech * 512 : (ech + 1) * 512],
            )

    def emit_tail_final(i):  # tiles 8-15, ct3 + yh partial
        ysb = yp.tile([128, E], BF16, tag="y")
        for ech in range(2):
            p = psF.tile([128, 512], F32, tag="f")
            nc.tensor.matmul(
                p,
                at_sb[:, 3, i * 128 : (i + 1) * 128],
                wp_sb[:, 3, ech * 512 : (ech + 1) * 512],
                start=True,
                stop=True,
            )
            # Pool cannot read PSUM on real HW, so odd lanes stage the psum
            # through an ACT copy (idle in the tail) and add on Pool; even
            # lanes are direct DVE adds — both chains run in parallel
            if (i + ech) % 2 == 0:
                nc.vector.tensor_add(
                    ysb[:, ech * 512 : (ech + 1) * 512],
                    p,
                    yh_sb[:, i - 8, ech * 512 : (ech + 1) * 512],
                )
            else:
                ss = stg.tile([128, 512], BF16, tag="st")
                nc.scalar.copy(out=ss, in_=p)
                nc.gpsimd.tensor_add(
                    ysb[:, ech * 512 : (ech + 1) * 512],
                    ss,
                    yh_sb[:, i - 8, ech * 512 : (ech + 1) * 512],
                )
        nc.sync.dma_start(out=out_d[i * 128 : (i + 1) * 128, :], in_=ysb)

    # ---- qkv + V emission, all up front at natural (early) priority.
    # The qk groups gate the exp chains (ACT critical path), so they should
    # execute as soon as their DMA deps land; the priority heap interleaves
    # the (tiny) attention score matmuls ahead of them as heads unlock.
    # Need-by order: qkA01 (c0 A heads), V0-7 (c0h0 bursts), qkA23 (c1h0),
    # V8-15 (c1h0 bursts), qkB01 (c0h4), qkB23 (c1h4). ----
    # blocks 0-3 tch0 feed the very first exps: emit as 256-col half
    # groups so the first scores unlock ~4us earlier on the DMA ramp
    for b in range(4):
        for qh in range(2):
            g, sl = BLK_GRP[b], BLK_SLAB[b]
            p = psF.tile([128, 256], F32, tag="f")
            for et in range(ET):
                nc.tensor.matmul(
                    p,
                    wqk_sb[:, b, et, :],
                    xt_sb[:, et, qh * 256 : (qh + 1) * 256],
                    start=(et == 0),
                    stop=(et == ET - 1),
                )
            nc.vector.tensor_scalar_add(
                qk8_sb[:, g, sl, qh * 256 : (qh + 1) * 256],
                p,
                bqk_sb[:, b : b + 1],
            )
    for b in range(4):
        emit_qk(b, 1)
    # c1h0's exps 0-7 need only the QA tch2/3 groups; its exps 8+ also
    # wait on c0-pass bursts (pt slot recycling) which need V-quadA 0-7,
    # so interleave: QA23, VqA0-7, KA23, VqA8-15
    for b in (0, 1):
        emit_qk(b, 2)
    for b in (0, 1):
        emit_qk(b, 3)
    for i in range(8):
        emit_v(i, 0)
    for b in (2, 3):
        emit_qk(b, 2)
    for b in (2, 3):
        emit_qk(b, 3)
    for i in range(8, 16):
        emit_v(i, 0)
    for tch in (0, 1):
        for b in range(4, 8):
            emit_qk(b, tch)
    for i in range(16):
        emit_v(i, 1)
    for tch in (2, 3):
        for b in range(4, 8):
            emit_qk(b, tch)
    late = [("cp", i) for i in range(8)]

    def run_fill(q, n):
        for _ in range(n):
            if not q:
                return
            it = q.pop(0)
            emit_cproj(it[1])

    # scores piece list: 256-col granularity (DoubleRow moving free <= 512),
    # one start/stop per 512-col psum bank
    def pieces(off):
        ps = []
        a = off
        while a < 1024:
            b = min((a // 256 + 1) * 256, 1024)
            ps.append((a, b))
            a = b
        first_in_bank, last_in_bank = {}, {}
        for idx, (a, b) in enumerate(ps):
            bank = a // 512
            first_in_bank.setdefault(bank, idx)
            last_in_bank[bank] = idx
        starts = set(first_in_bank.values())
        stops = set(last_in_bank.values())
        return ps, starts, stops

    # ---- attention: interleaved head-pass order. Chunk-0 passes are
    # PE-light/ACT-light, chunk-1 ACT-heavy; alternating them keeps ACT fed
    # continuously instead of a PE-bound first half + ACT-bound second ----
    passes = [
        (0, 0), (0, 1), (0, 2), (0, 3),
        (1, 0), (1, 1), (1, 2), (1, 3), (0, 4), (0, 5), (0, 6), (0, 7),
        (1, 4), (1, 5), (1, 6), (1, 7),
    ]
    an_tiles = {}
    gctr = 0  # global head-pass counter for pt slot rotation
    for pidx, (j, h) in enumerate(passes):
        q0 = j * 1024
        nkt = 8 * (j + 1)
        if True:
            m = h % 4
            gq = 0 if h < 4 else 2
            gk = 1 if h < 4 else 3
            sbase = (17 * gctr) % PTS
            gctr += 1
            hp = tc.high_priority(offset=PRIO_OFFSET)
            hp.__enter__()
            def emit_half(kt, ha, hb, wide):
                slot = (sbase + kt) % PTS
                off = max(0, (kt - 8 * j) * 128)
                if wide:
                    ps2 = psS.tile([128, 1024], F32, tag="sc")
                    pb = ps2[:, ha:hb]
                else:
                    ps2 = psS.tile([128, 512], F32, tag="sc")
                    pb = ps2[:, 0 : hb - ha]
                pcs, _, _ = pieces(off if wide else ha)
                pcs = [(a, b) for (a, b) in pcs if a >= ha and b <= hb]
                first_in_bank, last_in_bank = {}, {}
                for idx, (a, b) in enumerate(pcs):
                    bank = (a - (0 if wide else ha)) // 512
                    first_in_bank.setdefault(bank, idx)
                    last_in_bank[bank] = idx
                starts = set(first_in_bank.values())
                stops = set(last_in_bank.values())
                for idx, (a, b) in enumerate(pcs):
                    nc.tensor.matmul(
                        ps2[:, a:b] if wide else pb[:, a - ha : b - ha],
                        qk8_sb[
                            32 * m : 32 * m + 32, gk, :, kt * 128 : (kt + 1) * 128
                        ],
                        qk8_sb[32 * m : 32 * m + 32, gq, :, q0 + a : q0 + b],
                        start=(idx in starts),
                        stop=(idx in stops),
                        perf_mode=DR,
                        tile_position=(32 * m, 0),
                    )
                nc.scalar.activation(
                    out=pt_sb[:, slot, ha:hb],
                    in_=ps2[:, ha:hb] if wide else pb,
                    func=AF.Exp,
                    scale=0.125,
                )

            def emit_post(kt):
                # causal mask on the diagonal 128-block, then the AV burst
                # for q-block p, normalize, and (odd head) the transpose
                slot = (sbase + kt) % PTS
                p = kt - 8 * j
                off = p * 128
                nc.gpsimd.affine_select(
                    out=pt_sb[:, slot, off : off + 128],
                    in_=pt_sb[:, slot, off : off + 128],
                    compare_op=mybir.AluOpType.is_ge,
                    fill=0.0,
                    base=0,
                    pattern=[[1, 128]],
                    channel_multiplier=-1,
                )
                pa = psA.tile([128, 512], F32, tag="av")
                for kt2 in range(kt + 1):
                    sl2 = (sbase + kt2) % PTS
                    nc.tensor.matmul(
                        pa[:, 0:65],
                        pt_sb[:, sl2, off : off + 128],
                        va_sb[:, kt2, h * 65 : h * 65 + 65],
                        start=(kt2 == 0),
                        stop=(kt2 == kt),
                    )
                ri = rp.tile([128, 1], F32, tag="ri")
                nc.vector.reciprocal(ri, pa[:, 64:65])
                if h % 2 == 0:
                    anb = anp.tile([128, 128], BF16, tag="an")
                    an_tiles[(j, p)] = anb
                else:
                    anb = an_tiles[(j, p)]
                nc.vector.tensor_scalar_mul(
                    anb[:, (h % 2) * 64 : (h % 2) * 64 + 64], pa[:, 0:64], ri
                )
                if h % 2 == 1:
                    nc.sync.dma_start_transpose(
                        out=at_sb[:, h // 2, q0 + off : q0 + off + 128],
                        in_=anb,
                    )
                    if pidx == 15:
                        emit_tail_final(8 + p)

            if pidx == 0:
                # two-phase ramp pass: everything that only needs xt tch0
                # (q-cols 0:512) first, so ACT is not head-of-line blocked
                # behind tch1-dependent exp pieces while the DMA lands
                for kt in range(4):
                    off = kt * 128
                    for ha, hb in [(off, 256), (256, 512)]:
                        if hb > off:
                            emit_half(kt, max(ha, off), hb, False)
                    emit_post(kt)
                for kt in range(8):
                    emit_half(kt, max(512, kt * 128), 1024, False)
                    if kt >= 4:
                        emit_post(kt)
            else:
                for kt in range(nkt):
                    emit_half(kt, max(0, (kt - 8 * j) * 128), 1024, True)
                    if kt >= 8 * j:
                        emit_post(kt)
            hp.__exit__(None, None, None)
            # late fillers: c_proj tiles 0-7 (read chunk-0 A^T, complete
            # after the c0h7 pass)
            if pidx == 12:
                run_fill(late, 8)
            if pidx == 13:  # after c1h5: A^T ct0-2 rows complete for q>=1024
                for i in range(8, 12):
                    emit_tail_half(i)
            elif pidx == 14:
                for i in range(12, 16):
                    emit_tail_half(i)


def build_nc():
    _install_drain_fix()
    from contextlib import ExitStack

    nc = bacc.Bacc()
    with ExitStack() as ctx:
        tc = ctx.enter_context(tile.TileContext(nc))
        _emit(nc, tc, ctx)
    nc.finalize()  # Bacc: alloc_regs + insert_library_loads happen here
    return nc


def make_in_maps(inputs, w_attn, b_attn, w_proj, b_proj):
    """Build the 8 per-core input dicts from the full tensors.
    X / weights go down pre-converted to bf16 (the compute dtype); wqk
    columns are reordered into the fp8 paired-slab block layout."""
    x = np.asarray(inputs, dtype=np.float32)
    w_attn = np.asarray(w_attn, dtype=np.float32)
    b_attn = np.asarray(b_attn, dtype=np.float32)
    w_proj = np.asarray(w_proj, dtype=np.float32)
    b_proj = np.asarray(b_proj, dtype=np.float32)

    in_maps = []
    for c in range(8):
        b, half = c // 2, c % 2
        h0 = half * 8
        # block order QA0 QA1 KA0 KA1 QB0 QB1 KB0 KB1; each block = 4 heads
        # x 32 dims: [h(4) x slab-32]
        cols = []
        bqk = []
        for blk in range(8):
            g, sl = BLK_GRP[blk], BLK_SLAB[blk]
            qk_off = 0 if g in (0, 2) else 1024
            hbase = h0 + (0 if g in (0, 1) else 4)
            for hh in range(4):
                head = hbase + hh
                lo = qk_off + head * 64 + sl * 32
                cols.extend(range(lo, lo + 32))
        cols = np.array(cols)
        # [E, 1024] block-ordered -> [block, partition, et, col]
        wqk = np.ascontiguousarray(
            w_attn[:, cols]
            .reshape(8, 128, 8, 128)
            .transpose(2, 1, 0, 3)
            .astype(BF16_NP)
        )
        bqk = np.ascontiguousarray(b_attn[cols].reshape(8, 128).T)
        vbase = 2048 + h0 * 64
        wva = w_attn[:, vbase : vbase + 512]
        bva = b_attn[vbase : vbase + 512].reshape(1, 512)
        wp = np.ascontiguousarray(w_proj[h0 * 64 : h0 * 64 + 512, :].astype(BF16_NP))
        bp = (b_proj if half == 0 else np.zeros_like(b_proj)).reshape(1, E)
        in_maps.append(
            {
                "xt": np.ascontiguousarray(x[b].T.astype(BF16_NP)),
                "wqk": wqk,
                "wva": np.ascontiguousarray(wva.astype(BF16_NP)),
                "wp": wp,
                "bqk": np.ascontiguousarray(bqk.astype(np.float32)),
                "bva": bva.astype(BF16_NP),
                "bp": np.ascontiguousarray(bp.astype(BF16_NP)),
            }
        )
    return in_maps


_CACHE = {}


def kernel(**inputs):
    nc = _CACHE.get("nc")
    if nc is None:
        nc = _CACHE["nc"] = build_nc()
    in_maps = make_in_maps(
        inputs["inputs"],
        inputs["w_attn"],
        inputs["b_attn"],
        inputs["w_proj"],
        inputs["b_proj"],
    )
    res = run_bass_kernel_spmd(nc, in_maps, core_ids=list(range(8)))
    return gather(res.results)


def gather(results):
    out = np.zeros((4, S, E), dtype=np.float32)
    for b in range(4):
        for c in (2 * b, 2 * b + 1):
            out[b] += results[c]["out"].astype(np.float32)
    return out


# revision 72
# speedup vs baseline: 1.0018x; 1.0018x over previous
"""GPT-2 style causal attention block (B=4, S=2048, E=1024, H=16, D=64) on
8 TRN2 NeuronCores.

Sharding: batch(4) x head-half(2) -> 8 cores, zero on-device communication.
Core c handles batch b=c//2 and heads h0=(c%2)*8 .. h0+7. Each core computes
its qkv column block, attention for its 8 heads, and a partial c_proj
(its 512 rows of w_proj); core pairs are summed on the host.

Key structure (per core):
  X^T [E, S] host-pre-transposed bf16.
  Q,K are produced in fp8(e4m3) with a paired-slab layout [32, 2, S] per
  4-head group (w_attn columns reordered on the host so each psum group is
  one [128, 512] tile): scores run as fp8 DoubleRow matmuls at 0.5
  cycles/column, contraction d=64 as 2 slabs of 32.
  exp on ACT (scale 1/8 folded) into a rotating pt store [kt, q]; causal
  diagonal via gpsimd affine_select.
  attn@V is *flipped*: per (head, q-block) bursts with the pt block as the
  (free) stationary and [V_h | ones] as the 65-wide moving operand ->
  65 cycles per 128x128 block, denominator in column 64.
  Normalize on DVE (recip + per-partition scalar mul) into per-pair [128q,
  128hd] staging tiles, then DMA-transpose (xbar) into A^T [hd, q] - no PE
  or DVE cycles spent on the transpose.
  c_proj tiles 0-7 after the c0h7 pass; tiles 8-15 split ct0-2 (after
  c1h5) + ct3 per q-block inside the last pass, partials staged in bf16;
  the final adds alternate DVE and ACT-copy+Pool so no engine serializes
  the drain.

Scheduling: head passes are interleaved across the two q-chunks
(c0h0-3, c1h0, c1h1, c0h4, ...) so the ACT engine - the exp chain is its
critical path - stays fed from ~10us to the end instead of a PE-bound
first half and ACT-bound second half. All qkv/V groups are emitted up
front in ACT-unlock order (qkA01, qkA23, V-quadA, qkB01, V-quadB, qkB23);
attention bodies are priority-boosted over them. pt slots rotate by 17
(mod 24) per pass so a pass's exps only collide with bursts >=1 pass
back. PSUM: scores 2x2 banks, AV bursts 2x1, fillers 2x1.
"""

import re

import ml_dtypes
import numpy as np

import concourse.mybir as mybir
import concourse.tile as tile
from concourse import bacc
from concourse.bass_utils import run_bass_kernel_spmd
from concourse.vector_clock import ScopedClock

F32 = mybir.dt.float32
BF16 = mybir.dt.bfloat16
FP8 = mybir.dt.float8e4
BF16_NP = ml_dtypes.bfloat16
FP8_NP = ml_dtypes.float8_e4m3fn
AF = mybir.ActivationFunctionType
DR = mybir.MatmulPerfMode.DoubleRow

S = 2048          # sequence length (per batch)
E = 1024          # embedding dim
HL = 8            # heads per core
D = 64            # head dim
TT = S // 128     # 16 token tiles
ET = E // 128     # 8 embedding tiles
NCH = S // 1024   # 2 q-chunks of 1024
PTS = 24          # pt slot count; advance 17 per pass (coprime) so adjacent
                  # passes never collide and older overlaps spread thin
PRIO_OFFSET = 800  # attention body scheduled ahead of filler work

# wqk block order (host column layout): QA0 QA1 KA0 KA1 QB0 QB1 KB0 KB1
# grp: 0=QA 1=KA 2=QB 3=KB ; slab: d 0:32 / 32:64 per head
BLK_GRP = [0, 0, 1, 1, 2, 2, 3, 3]
BLK_SLAB = [0, 1, 0, 1, 0, 1, 0, 1]


def _install_drain_fix():
    """walrus in this container rejects the Tile kernel-tail Drain when it
    carries all semaphore waits on one instruction ("Too many sync wait
    commands"). Emit one wait_ge per semaphore, then a bare drain."""
    if getattr(tile.TileContext, "_drain_fix_installed", False):
        return

    def _split_drain_and_barrier(self, tick_clock, wait_clock):
        nc = self.nc
        probe = mybir.InstDrain(
            name="probe-drain", engine=mybir.EngineType.SP, ins=[], outs=[]
        )
        wait_clock.add_sem_waits(probe, ScopedClock({None: tick_clock.global_clock}))
        waits = re.findall(r"wait:S\[([A-Za-z0-9_]+)\]>=(\d+)", probe.concise())
        handles = {h.name: h for h in self.sems.allocated().values()}
        for name, val in waits:
            nc.sync.wait_ge(handles[name], int(val))
        nc.sync.drain()
        nc.all_engine_barrier()
        popped = nc._tile_sem_poison_stack.pop()
        assert popped is self._sem_poison
        nc.clear_and_free_semaphores(list(self.sems.allocated().values()))
        nc.all_engine_barrier()

    tile.TileContext._drain_and_barrier = _split_drain_and_barrier
    tile.TileContext._drain_fix_installed = True


def _emit(nc, tc, ctx):
    xt_d = nc.declare_dram_parameter("xt", [E, S], BF16, isOutput=False)
    # wqk host-side block-major: [block, partition, et, col] so one block =
    # one contiguous 256KB DMA with 2KB descriptor lines
    wqk_d = nc.declare_dram_parameter("wqk", [8, 128, ET, 128], BF16, isOutput=False)
    wva_d = nc.declare_dram_parameter("wva", [E, 512], BF16, isOutput=False)
    wp_d = nc.declare_dram_parameter("wp", [512, E], BF16, isOutput=False)
    bqk_d = nc.declare_dram_parameter("bqk", [128, 8], F32, isOutput=False)
    bva_d = nc.declare_dram_parameter("bva", [1, 512], BF16, isOutput=False)
    bp_d = nc.declare_dram_parameter("bp", [1, E], BF16, isOutput=False)
    out_d = nc.declare_dram_parameter("out", [S, E], BF16, isOutput=True)

    consts = ctx.enter_context(tc.tile_pool(name="consts", bufs=1))
    statics = ctx.enter_context(tc.tile_pool(name="statics", bufs=1))
    anp = ctx.enter_context(tc.tile_pool(name="anp", bufs=20))
    rp = ctx.enter_context(tc.tile_pool(name="rp", bufs=4))
    yp = ctx.enter_context(tc.tile_pool(name="yp", bufs=3))
    stg = ctx.enter_context(tc.tile_pool(name="stg", bufs=2))
    # PSUM budget (8 banks): scores 2x[128,1024]=4, AV 2x[128,512(:65)]=2,
    # fillers 2x[128,512]=2
    psS = ctx.enter_context(tc.tile_pool(name="psS", bufs=2, space="PSUM"))
    psA = ctx.enter_context(tc.tile_pool(name="psA", bufs=2, space="PSUM"))
    psF = ctx.enter_context(tc.tile_pool(name="psF", bufs=2, space="PSUM"))

    # ---- static SBUF tiles ----
    xt_sb = statics.tile([128, ET, S], BF16)
    wqk_sb = statics.tile([128, 8, ET, 128], BF16)
    wva_sb = statics.tile([128, ET, 512], BF16)
    wp_sb = statics.tile([128, 4, E], BF16)
    # Q,K fp8 paired-slab store: [part 32m, grp, slab, col]
    qk8_sb = statics.tile([128, 4, 2, S], FP8)
    va_sb = statics.tile([128, TT, HL * 65], BF16)
    pt_sb = statics.tile([128, PTS, 1024], BF16)
    at_sb = statics.tile([128, 4, S], BF16)  # A^T: rows c=h*64+d, cols t
    yh_sb = statics.tile([128, 8, E], BF16)  # tail (tiles 8-15) ct0-2 partial

    # ---- front DMA: the DMA engines are a single serialized ~360GB/s
    # resource and issue overhead is per-queue, so spread across three
    # queues: SP carries xt (tch-major pieces so chunk-0 unlocks first),
    # ACT (idle during the ramp) carries the wqk blocks, Pool carries
    # wva/wp. Biases first (tiny, gate the qk8 copies). ----
    bqk_sb = consts.tile([128, 8], F32)
    nc.sync.dma_start(out=bqk_sb, in_=bqk_d[:])
    for b in range(4):
        nc.scalar.dma_start(out=wqk_sb[:, b], in_=wqk_d[b])

    for et in range(ET):
        nc.sync.dma_start(
            out=xt_sb[:, et, 0:512], in_=xt_d[et * 128 : (et + 1) * 128, 0:512]
        )
    for et in range(ET):
        nc.sync.dma_start(
            out=xt_sb[:, et, 512:1024],
            in_=xt_d[et * 128 : (et + 1) * 128, 512:1024],
        )
    for et in range(ET):
        nc.sync.dma_start(
            out=xt_sb[:, et, 1024:1536],
            in_=xt_d[et * 128 : (et + 1) * 128, 1024:1536],
        )
    for et in range(ET):
        nc.gpsimd.dma_start(
            out=wva_sb[:, et, :], in_=wva_d[et * 128 : (et + 1) * 128, :]
        )
    for b in range(4, 8):
        nc.gpsimd.dma_start(out=wqk_sb[:, b], in_=wqk_d[b])
    bva_st = consts.tile([1, 512], BF16)
    nc.gpsimd.dma_start(out=bva_st, in_=bva_d[:])
    bva_bc = consts.tile([128, 512], BF16)
    nc.gpsimd.partition_broadcast(out_ap=bva_bc[:], in_ap=bva_st[:])
    bp_st = consts.tile([1, E], BF16)
    nc.gpsimd.dma_start(out=bp_st, in_=bp_d[:])
    bp_bc = consts.tile([128, E], BF16)
    nc.gpsimd.partition_broadcast(out_ap=bp_bc[:], in_ap=bp_st[:])
    for et in range(ET):
        nc.sync.dma_start(
            out=xt_sb[:, et, 1536:2048],
            in_=xt_d[et * 128 : (et + 1) * 128, 1536:2048],
        )
    for ct in range(4):
        nc.gpsimd.dma_start(out=wp_sb[:, ct, :], in_=wp_d[ct * 128 : (ct + 1) * 128, :])

    # per-head ones columns (position 64 of each 65-wide head block)
    nc.gpsimd.memset(
        va_sb[:, :, :].rearrange("p i (h c) -> p i h c", c=65)[:, :, :, D : D + 1],
        1.0,
    )

    # ---- qkv emitters ----
    def emit_qk(b, tch):
        g, sl = BLK_GRP[b], BLK_SLAB[b]
        p = psF.tile([128, 512], F32, tag="f")
        for et in range(ET):
            nc.tensor.matmul(
                p,
                wqk_sb[:, b, et, :],
                xt_sb[:, et, tch * 512 : (tch + 1) * 512],
                start=(et == 0),
                stop=(et == ET - 1),
            )
        nc.vector.tensor_scalar_add(
            qk8_sb[:, g, sl, tch * 512 : (tch + 1) * 512], p, bqk_sb[:, b : b + 1]
        )

    def emit_v(i, q):  # token tile i, head-quad q (heads 4q..4q+3)
        p = psF.tile([128, 512], F32, tag="f")
        for et in range(ET):
            nc.tensor.matmul(
                p[:, 0:256],
                xt_sb[:, et, i * 128 : (i + 1) * 128],
                wva_sb[:, et, q * 256 : (q + 1) * 256],
                start=(et == 0),
                stop=(et == ET - 1),
            )
        nc.vector.tensor_add(
            va_sb[:, i, 260 * q : 260 * (q + 1)].rearrange(
                "p (h c) -> p h c", c=65
            )[:, :, 0:D],
            p[:, 0:256].rearrange("p (h c) -> p h c", c=D),
            bva_bc[:, q * 256 : (q + 1) * 256].rearrange("p (h c) -> p h c", c=D),
        )

    def emit_cproj(i):  # tiles 0-7: full ct accumulation
        ysb = yp.tile([128, E], BF16, tag="y")
        for ech in range(2):
            p = psF.tile([128, 512], F32, tag="f")
            for ct in range(4):
                nc.tensor.matmul(
                    p,
                    at_sb[:, ct, i * 128 : (i + 1) * 128],
                    wp_sb[:, ct, ech * 512 : (ech + 1) * 512],
                    start=(ct == 0),
                    stop=(ct == 3),
                )
            nc.vector.tensor_add(
                ysb[:, ech * 512 : (ech + 1) * 512],
                p,
                bp_bc[:, ech * 512 : (ech + 1) * 512],
            )
        nc.sync.dma_start(out=out_d[i * 128 : (i + 1) * 128, :], in_=ysb)

    def emit_tail_half(i):  # tiles 8-15, ct 0..2 (+bias) into yh_sb
        for ech in range(2):
            p = psF.tile([128, 512], F32, tag="f")
            for ct in range(3):
                nc.tensor.matmul(
                    p,
                    at_sb[:, ct, i * 128 : (i + 1) * 128],
                    wp_sb[:, ct, ech * 512 : (ech + 1) * 512],
                    start=(ct == 0),
                    stop=(ct == 2),
                )
            nc.vector.tensor_add(
                yh_sb[:, i - 8, ech * 512 : (ech + 1) * 512],
                p,
                bp_bc[:, ech * 512 : (ech + 1) * 512],
            )

    def emit_tail_final(i):  # tiles 8-15, ct3 + yh partial
        ysb = yp.tile([128, E], BF16, tag="y")
        for ech in range(2):
            p = psF.tile([128, 512], F32, tag="f")
            nc.tensor.matmul(
                p,
                at_sb[:, 3, i * 128 : (i + 1) * 128],
                wp_sb[:, 3, ech * 512 : (ech + 1) * 512],
                start=True,
                stop=True,
            )
            # Pool cannot read PSUM on real HW, so odd lanes stage the psum
            # through an ACT copy (idle in the tail) and add on Pool; even
            # lanes are direct DVE adds — both chains run in parallel. The
            # last two tiles are the drain's critical chain: keep them on
            # the low-latency DVE path and stream each half out as it lands.
            if i >= 14 or (i + ech) % 2 == 0:
                nc.vector.tensor_add(
                    ysb[:, ech * 512 : (ech + 1) * 512],
                    p,
                    yh_sb[:, i - 8, ech * 512 : (ech + 1) * 512],
                )
            else:
                ss = stg.tile([128, 512], BF16, tag="st")
                nc.scalar.copy(out=ss, in_=p)
                nc.gpsimd.tensor_add(
                    ysb[:, ech * 512 : (ech + 1) * 512],
                    ss,
                    yh_sb[:, i - 8, ech * 512 : (ech + 1) * 512],
                )
            if i >= 14:
                nc.sync.dma_start(
                    out=out_d[
                        i * 128 : (i + 1) * 128, ech * 512 : (ech + 1) * 512
                    ],
                    in_=ysb[:, ech * 512 : (ech + 1) * 512],
                )
        if i < 14:
            nc.sync.dma_start(out=out_d[i * 128 : (i + 1) * 128, :], in_=ysb)

    # ---- qkv + V emission, all up front at natural (early) priority.
    # The qk groups gate the exp chains (ACT critical path), so they should
    # execute as soon as their DMA deps land; the priority heap interleaves
    # the (tiny) attention score matmuls ahead of them as heads unlock.
    # Need-by order: qkA01 (c0 A heads), V0-7 (c0h0 bursts), qkA23 (c1h0),
    # V8-15 (c1h0 bursts), qkB01 (c0h4), qkB23 (c1h4). ----
    # blocks 0-3 tch0 feed the very first exps: emit as 256-col half
    # groups so the first scores unlock ~4us earlier on the DMA ramp
    for b in range(4):
        for qh in range(2):
            g, sl = BLK_GRP[b], BLK_SLAB[b]
            p = psF.tile([128, 256], F32, tag="f")
            for et in range(ET):
                nc.tensor.matmul(
                    p,
                    wqk_sb[:, b, et, :],
                    xt_sb[:, et, qh * 256 : (qh + 1) * 256],
                    start=(et == 0),
                    stop=(et == ET - 1),
                )
            nc.vector.tensor_scalar_add(
                qk8_sb[:, g, sl, qh * 256 : (qh + 1) * 256],
                p,
                bqk_sb[:, b : b + 1],
            )
    for b in range(4):
        emit_qk(b, 1)
    # c1h0's exps 0-7 need only the QA tch2/3 groups; its exps 8+ also
    # wait on c0-pass bursts (pt slot recycling) which need V-quadA 0-7,
    # so interleave: QA23, VqA0-7, KA23, VqA8-15
    for b in (0, 1):
        emit_qk(b, 2)
    for b in (0, 1):
        emit_qk(b, 3)
    for i in range(8):
        emit_v(i, 0)
    for b in (2, 3):
        emit_qk(b, 2)
    for b in (2, 3):
        emit_qk(b, 3)
    for i in range(8, 16):
        emit_v(i, 0)
    for tch in (0, 1):
        for b in range(4, 8):
            emit_qk(b, tch)
    for i in range(16):
        emit_v(i, 1)
    for tch in (2, 3):
        for b in range(4, 8):
            emit_qk(b, tch)
    late = [("cp", i) for i in range(8)]

    def run_fill(q, n):
        for _ in range(n):
            if not q:
                return
            it = q.pop(0)
            emit_cproj(it[1])

    # scores piece list: 256-col granularity (DoubleRow moving free <= 512),
    # one start/stop per 512-col psum bank
    def pieces(off):
        ps = []
        a = off
        while a < 1024:
            b = min((a // 256 + 1) * 256, 1024)
            ps.append((a, b))
            a = b
        first_in_bank, last_in_bank = {}, {}
        for idx, (a, b) in enumerate(ps):
            bank = a // 512
            first_in_bank.setdefault(bank, idx)
            last_in_bank[bank] = idx
        starts = set(first_in_bank.values())
        stops = set(last_in_bank.values())
        return ps, starts, stops

    # ---- attention: interleaved head-pass order. Chunk-0 passes are
    # PE-light/ACT-light, chunk-1 ACT-heavy; alternating them keeps ACT fed
    # continuously instead of a PE-bound first half + ACT-bound second ----
    passes = [
        (0, 0), (0, 1), (0, 2), (0, 3),
        (1, 0), (1, 1), (1, 2), (1, 3), (0, 4), (0, 5), (0, 6), (0, 7),
        (1, 4), (1, 5), (1, 6), (1, 7),
    ]
    an_tiles = {}
    gctr = 0  # global head-pass counter for pt slot rotation
    for pidx, (j, h) in enumerate(passes):
        q0 = j * 1024
        nkt = 8 * (j + 1)
        if True:
            m = h % 4
            gq = 0 if h < 4 else 2
            gk = 1 if h < 4 else 3
            sbase = (17 * gctr) % PTS
            gctr += 1
            hp = tc.high_priority(offset=PRIO_OFFSET)
            hp.__enter__()
            def emit_half(kt, ha, hb, wide):
                slot = (sbase + kt) % PTS
                off = max(0, (kt - 8 * j) * 128)
                if wide:
                    ps2 = psS.tile([128, 1024], F32, tag="sc")
                    pb = ps2[:, ha:hb]
                else:
                    ps2 = psS.tile([128, 512], F32, tag="sc")
                    pb = ps2[:, 0 : hb - ha]
                pcs, _, _ = pieces(off if wide else ha)
                pcs = [(a, b) for (a, b) in pcs if a >= ha and b <= hb]
                first_in_bank, last_in_bank = {}, {}
                for idx, (a, b) in enumerate(pcs):
                    bank = (a - (0 if wide else ha)) // 512
                    first_in_bank.setdefault(bank, idx)
                    last_in_bank[bank] = idx
                starts = set(first_in_bank.values())
                stops = set(last_in_bank.values())
                for idx, (a, b) in enumerate(pcs):
                    nc.tensor.matmul(
                        ps2[:, a:b] if wide else pb[:, a - ha : b - ha],
                        qk8_sb[
                            32 * m : 32 * m + 32, gk, :, kt * 128 : (kt + 1) * 128
                        ],
                        qk8_sb[32 * m : 32 * m + 32, gq, :, q0 + a : q0 + b],
                        start=(idx in starts),
                        stop=(idx in stops),
                        perf_mode=DR,
                        tile_position=(32 * m, 0),
                    )
                nc.scalar.activation(
                    out=pt_sb[:, slot, ha:hb],
                    in_=ps2[:, ha:hb] if wide else pb,
                    func=AF.Exp,
                    scale=0.125,
                )

            def emit_post(kt):
                # causal mask on the diagonal 128-block, then the AV burst
                # for q-block p, normalize, and (odd head) the transpose
                slot = (sbase + kt) % PTS
                p = kt - 8 * j
                off = p * 128
                nc.gpsimd.affine_select(
                    out=pt_sb[:, slot, off : off + 128],
                    in_=pt_sb[:, slot, off : off + 128],
                    compare_op=mybir.AluOpType.is_ge,
                    fill=0.0,
                    base=0,
                    pattern=[[1, 128]],
                    channel_multiplier=-1,
                )
                pa = psA.tile([128, 512], F32, tag="av")
                for kt2 in range(kt + 1):
                    sl2 = (sbase + kt2) % PTS
                    nc.tensor.matmul(
                        pa[:, 0:65],
                        pt_sb[:, sl2, off : off + 128],
                        va_sb[:, kt2, h * 65 : h * 65 + 65],
                        start=(kt2 == 0),
                        stop=(kt2 == kt),
                    )
                ri = rp.tile([128, 1], F32, tag="ri")
                nc.vector.reciprocal(ri, pa[:, 64:65])
                if h % 2 == 0:
                    anb = anp.tile([128, 128], BF16, tag="an")
                    an_tiles[(j, p)] = anb
                else:
                    anb = an_tiles[(j, p)]
                nc.vector.tensor_scalar_mul(
                    anb[:, (h % 2) * 64 : (h % 2) * 64 + 64], pa[:, 0:64], ri
                )
                if h % 2 == 1:
                    nc.sync.dma_start_transpose(
                        out=at_sb[:, h // 2, q0 + off : q0 + off + 128],
                        in_=anb,
                    )
                    if pidx == 15:
                        emit_tail_final(8 + p)

            if pidx == 0:
                # two-phase ramp pass: everything that only needs xt tch0
                # (q-cols 0:512) first, so ACT is not head-of-line blocked
                # behind tch1-dependent exp pieces while the DMA lands
                for kt in range(4):
                    off = kt * 128
                    for ha, hb in [(off, 256), (256, 512)]:
                        if hb > off:
                            emit_half(kt, max(ha, off), hb, False)
                    emit_post(kt)
                for kt in range(8):
                    emit_half(kt, max(512, kt * 128), 1024, False)
                    if kt >= 4:
                        emit_post(kt)
            else:
                for kt in range(nkt):
                    emit_half(kt, max(0, (kt - 8 * j) * 128), 1024, True)
                    if kt >= 8 * j:
                        emit_post(kt)
            hp.__exit__(None, None, None)
            # late fillers: c_proj tiles 0-7 (read chunk-0 A^T, complete
            # after the c0h7 pass)
            if pidx == 12:
                run_fill(late, 8)
            if pidx == 13:  # after c1h5: A^T ct0-2 rows complete for q>=1024
                for i in range(8, 12):
                    emit_tail_half(i)
            elif pidx == 14:
                for i in range(12, 16):
                    emit_tail_half(i)


def build_nc():
    _install_drain_fix()
    from contextlib import ExitStack

    nc = bacc.Bacc()
    with ExitStack() as ctx:
        tc = ctx.enter_context(tile.TileContext(nc))
        _emit(nc, tc, ctx)
    nc.finalize()  # Bacc: alloc_regs + insert_library_loads happen here
    return nc


def make_in_maps(inputs, w_attn, b_attn, w_proj, b_proj):
    """Build the 8 per-core input dicts from the full tensors.
    X / weights go down pre-converted to bf16 (the compute dtype); wqk
    columns are reordered into the fp8 paired-slab block layout."""
    x = np.asarray(inputs, dtype=np.float32)
    w_attn = np.asarray(w_attn, dtype=np.float32)
    b_attn = np.asarray(b_attn, dtype=np.float32)
    w_proj = np.asarray(w_proj, dtype=np.float32)
    b_proj = np.asarray(b_proj, dtype=np.float32)

    in_maps = []
    for c in range(8):
        b, half = c // 2, c % 2
        h0 = half * 8
        # block order QA0 QA1 KA0 KA1 QB0 QB1 KB0 KB1; each block = 4 heads
        # x 32 dims: [h(4) x slab-32]
        cols = []
        bqk = []
        for blk in range(8):
            g, sl = BLK_GRP[blk], BLK_SLAB[blk]
            qk_off = 0 if g in (0, 2) else 1024
            hbase = h0 + (0 if g in (0, 1) else 4)
            for hh in range(4):
                head = hbase + hh
                lo = qk_off + head * 64 + sl * 32
                cols.extend(range(lo, lo + 32))
        cols = np.array(cols)
        # [E, 1024] block-ordered -> [block, partition, et, col]
        wqk = np.ascontiguousarray(
            w_attn[:, cols]
            .reshape(8, 128, 8, 128)
            .transpose(2, 1, 0, 3)
            .astype(BF16_NP)
        )
        bqk = np.ascontiguousarray(b_attn[cols].reshape(8, 128).T)
        vbase = 2048 + h0 * 64
        wva = w_attn[:, vbase : vbase + 512]
        bva = b_attn[vbase : vbase + 512].reshape(1, 512)
        wp = np.ascontiguousarray(w_proj[h0 * 64 : h0 * 64 + 512, :].astype(BF16_NP))
        bp = (b_proj if half == 0 else np.zeros_like(b_proj)).reshape(1, E)
        in_maps.append(
            {
                "xt": np.ascontiguousarray(x[b].T.astype(BF16_NP)),
                "wqk": wqk,
                "wva": np.ascontiguousarray(wva.astype(BF16_NP)),
                "wp": wp,
                "bqk": np.ascontiguousarray(bqk.astype(np.float32)),
                "bva": bva.astype(BF16_NP),
                "bp": np.ascontiguousarray(bp.astype(BF16_NP)),
            }
        )
    return in_maps


_CACHE = {}


def kernel(**inputs):
    nc = _CACHE.get("nc")
    if nc is None:
        nc = _CACHE["nc"] = build_nc()
    in_maps = make_in_maps(
        inputs["inputs"],
        inputs["w_attn"],
        inputs["b_attn"],
        inputs["w_proj"],
        inputs["b_proj"],
    )
    res = run_bass_kernel_spmd(nc, in_maps, core_ids=list(range(8)))
    return gather(res.results)


def gather(results):
    out = np.zeros((4, S, E), dtype=np.float32)
    for b in range(4):
        for c in (2 * b, 2 * b + 1):
            out[b] += results[c]["out"].astype(np.float32)
    return out
